# revision 1
# baseline (speedup 1.0000x reference)
"""Trainium2 Bass kernel for nn_BagModel_3d (segment_reduce).

Computation (per bag b):
  out[b] = (1/n_b) * sum_{i < n_b} relu(x[b, i, :] @ W1 + b1) @ W2 + b2

Strategy: data-parallel over bags, 32 bags per NeuronCore across 8 cores.
Host-side prep is layout only: shard x on the bag axis, transpose each shard
to [D_IN, bags*N_MAX] so the contraction dim lands on SBUF partitions, and
zero the padding instances (i >= n_b).

Per core, per (bag, dh-chunk): a [128, 512] PSUM tile accumulates the two
K=128 halves of z = x @ W1 (bf16 datapath, fp32 PSUM). The PSUM drain —
relu(z + b1) with a fused free-axis accumulation (the segment sum) — runs in
ONE instruction, alternating between ScalarE (activation+accum) and VectorE
(scalar_tensor_tensor+accum) so neither engine paces the loop. Zeroed
padding contributes relu(b1) per padded instance; a rank-1 (n_b-512) x
relu(b1) correction (exactly 0 for the spec's b1=0) restores the true sum.
The mean's 1/n and +b2 fold into one per-partition op on the final [32,1]
PSUM output of the W2 matmul.
"""
import sys
import numpy as np

sys.path.insert(0, '/opt/trn_rl_repo')

B, N_MAX, D_IN, D_H = 256, 512, 256, 256
N_CORES = 8
BAGS = B // N_CORES          # 32 bags per core
R = BAGS * N_MAX             # 16384 instance rows per core
GROUPS = 8                   # bag groups per core (4 bags each)
GB = BAGS // GROUPS          # bags per group = 4
GW = GB * N_MAX              # row width per group = 2048

_PROGRAM = None
_PROGRAM_KEY = None


def _build_program(b2_value):
    import concourse.bacc as bacc
    import concourse.tile as tile
    from concourse import mybir

    f32 = mybir.dt.float32
    bf16 = mybir.dt.bfloat16
    i32 = mybir.dt.int32
    Alu = mybir.AluOpType

    nc = bacc.Bacc("TRN2", target_bir_lowering=False, debug=False)

    xt = nc.dram_tensor("xt", [D_IN, R], f32, kind="ExternalInput").ap()
    n_col = nc.dram_tensor("n_col", [BAGS, 1], i32, kind="ExternalInput").ap()
    n_row = nc.dram_tensor("n_row", [1, BAGS], i32, kind="ExternalInput").ap()
    w1 = nc.dram_tensor("w1", [D_IN, D_H], f32, kind="ExternalInput").ap()
    b1 = nc.dram_tensor("b1", [D_H, 1], f32, kind="ExternalInput").ap()
    w2 = nc.dram_tensor("w2", [D_H, 1], f32, kind="ExternalInput").ap()
    out = nc.dram_tensor("out", [BAGS, 1], f32, kind="ExternalOutput").ap()

    with tile.TileContext(nc) as tc:
        with (
            tc.tile_pool(name="const", bufs=1) as cpool,
            tc.tile_pool(name="x", bufs=8) as xpool,
            tc.tile_pool(name="h", bufs=4) as hpool,
            tc.tile_pool(name="z", bufs=5, space="PSUM") as zpool,
            tc.tile_pool(name="smallps", bufs=1, space="PSUM") as spspool,
            tc.tile_pool(name="xf", bufs=2) as xf_pool,
        ):
            # ---- weights first (first matmul needs them), then x prefetch ----
            w1k0 = cpool.tile([128, D_H], bf16, tag="w1k0")
            w1k1 = cpool.tile([128, D_H], bf16, tag="w1k1")
            nc.gpsimd.dma_start(w1k0[:], w1[0:128, :])   # SWDGE f32->bf16 cast
            nc.gpsimd.dma_start(w1k1[:], w1[128:256, :])
            # Split the 16 x loads across both DGE paths: SWDGE casts f32->bf16
            # inline but its descriptor rings contend on SDMA engines 7/15;
            # HWDGE loads f32 (no ring pressure) and ACT/DVE cast on-chip.
            xtiles = []
            for g in range(GROUPS):
                pair = []
                for k in range(2):
                    xbf = xpool.tile([128, GW], bf16, tag=f"x{k}")
                    src = xt[128 * k:128 * (k + 1), GW * g:GW * (g + 1)]
                    if g % 2 == 0:
                        nc.gpsimd.dma_start(xbf[:], src)
                    else:
                        xf = xf_pool.tile([128, GW], f32, tag=f"xf{k}")
                        nc.sync.dma_start(xf[:], src)
                        if k == 0:
                            nc.scalar.copy(xbf[:], xf[:])
                        else:
                            nc.vector.tensor_copy(xbf[:], xf[:])
                    pair.append(xbf)
                xtiles.append(tuple(pair))
            b1t = cpool.tile([128, 2], f32, tag="b1t")
            nc.sync.dma_start(b1t[:, 0:1], b1[0:128, :])
            nc.sync.dma_start(b1t[:, 1:2], b1[128:256, :])
            w2t = cpool.tile([128, 2], f32, tag="w2t")
            nc.sync.dma_start(w2t[:, 0:1], w2[0:128, :])
            nc.sync.dma_start(w2t[:, 1:2], w2[128:256, :])
            zeros_t = cpool.tile([128, N_MAX], f32, tag="zeros_t")
            nc.vector.memset(zeros_t[:], 0.0)

            # ---- n-derived scalars ----
            nI_col = cpool.tile([BAGS, 1], i32, tag="nI_col")
            nc.sync.dma_start(nI_col[:], n_col[:])
            nf_col = cpool.tile([BAGS, 1], f32, tag="nf_col")
            nc.vector.tensor_copy(nf_col[:], nI_col[:])
            inv_col = cpool.tile([BAGS, 1], f32, tag="inv_col")
            nc.vector.reciprocal(inv_col[:], nf_col[:])

            # padding correction: corr_c = relu(b1_c) (x) (n - 512)  [128, BAGS]
            nI_row = cpool.tile([1, BAGS], i32, tag="nI_row")
            nc.sync.dma_start(nI_row[:], n_row[:])
            nf_row = cpool.tile([1, BAGS], f32, tag="nf_row")
            nc.vector.tensor_copy(nf_row[:], nI_row[:])
            cnt_row = cpool.tile([1, BAGS], f32, tag="cnt_row")
            nc.vector.tensor_scalar(cnt_row[:], nf_row[:], 512.0, None,
                                    op0=Alu.subtract)
            b1row = cpool.tile([1, D_H], f32, tag="b1row")
            nc.sync.dma_start(b1row[:], b1[:, :])
            rb1row = cpool.tile([1, D_H], f32, tag="rb1row")
            nc.vector.tensor_scalar(rb1row[:], b1row[:], 0.0, None, op0=Alu.max)

            praw0 = cpool.tile([128, BAGS], f32, tag="praw0")
            praw1 = cpool.tile([128, BAGS], f32, tag="praw1")
            praws = (praw0, praw1)

            # ---- main loop ----
            for g in range(GROUPS):
                x0, x1 = xtiles[g]
                for j in range(GB):
                    b = GB * g + j
                    for c in range(2):
                        z = zpool.tile([128, N_MAX], f32, tag="z")
                        nc.tensor.matmul(
                            z[:], w1k0[:, 128 * c:128 * (c + 1)],
                            x0[:, N_MAX * j:N_MAX * (j + 1)],
                            start=True, stop=False)
                        nc.tensor.matmul(
                            z[:], w1k1[:, 128 * c:128 * (c + 1)],
                            x1[:, N_MAX * j:N_MAX * (j + 1)],
                            start=False, stop=True)
                        h = hpool.tile([128, N_MAX], f32, tag="h")
                        if c == 0:
                            # ScalarE: relu(z + b1) with fused row-sum
                            nc.scalar.activation(
                                h[:], z[:], mybir.ActivationFunctionType.Relu,
                                bias=b1t[:, c:c + 1], scale=1.0,
                                accum_out=praws[c][:, b:b + 1])
                        else:
                            # VectorE: max(z + b1, 0) with fused row-sum
                            nc.vector.scalar_tensor_tensor(
                                h[:], z[:], b1t[:, c:c + 1], zeros_t[:],
                                op0=Alu.add, op1=Alu.max,
                                accum_out=praws[c][:, b:b + 1])

            # ---- padding correction + final Linear ----
            pscs = []
            for c in range(2):
                corr = spspool.tile([128, BAGS], f32, tag=f"corr{c}")
                nc.tensor.matmul(corr[:], rb1row[0:1, 128 * c:128 * (c + 1)],
                                 cnt_row[:], start=True, stop=True)
                psc = cpool.tile([128, BAGS], f32, tag=f"psc{c}")
                nc.vector.tensor_add(psc[:], praws[c][:], corr[:])
                pscs.append(psc)
            po = spspool.tile([BAGS, 1], f32, tag="po")
            nc.tensor.matmul(po[:], pscs[0][:], w2t[:, 0:1], start=True, stop=False)
            nc.tensor.matmul(po[:], pscs[1][:], w2t[:, 1:2], start=False, stop=True)
            osb = cpool.tile([BAGS, 1], f32, tag="osb")
            nc.vector.tensor_scalar(
                osb[:], po[:], inv_col[:, 0:1], float(b2_value),
                op0=Alu.mult, op1=Alu.add)
            nc.sync.dma_start(out[:], osb[:])

    nc.compile()
    return nc


def get_program(b2_value=0.0):
    global _PROGRAM, _PROGRAM_KEY
    key = float(b2_value)
    if _PROGRAM is None or _PROGRAM_KEY != key:
        _PROGRAM = _build_program(key)
        _PROGRAM_KEY = key
    return _PROGRAM


def make_in_maps(x, n_instances, W1, b1, W2, b2=None):
    x = np.asarray(x, dtype=np.float32)
    n = np.asarray(n_instances, dtype=np.int32)
    W1 = np.asarray(W1, dtype=np.float32)
    b1 = np.asarray(b1, dtype=np.float32).reshape(D_H, 1)
    W2 = np.asarray(W2, dtype=np.float32).reshape(D_H, 1)
    in_maps = []
    for c in range(N_CORES):
        xs = x[c * BAGS:(c + 1) * BAGS]              # [32, 512, 256]
        xt = np.ascontiguousarray(xs.transpose(2, 0, 1).reshape(D_IN, R))
        ns = n[c * BAGS:(c + 1) * BAGS]
        for i in range(BAGS):                        # zero padding instances
            xt[:, i * N_MAX + int(ns[i]):(i + 1) * N_MAX] = 0.0
        in_maps.append({
            "xt": xt,
            "n_col": np.ascontiguousarray(ns.reshape(BAGS, 1)),
            "n_row": np.ascontiguousarray(ns.reshape(1, BAGS)),
            "w1": W1, "b1": b1, "w2": W2,
        })
    return in_maps


def run_spmd(in_maps, b2_value=0.0, trace=False, **kwargs):
    from concourse import bass_utils
    if trace:
        # no S3 in this environment; keep trace artifacts local
        bass_utils.upload_artifacts = lambda tmpdir: tmpdir
    nc = get_program(b2_value)
    return bass_utils.run_bass_kernel_spmd(
        nc, in_maps, core_ids=list(range(N_CORES)), trace=trace, **kwargs)


def kernel(x, n_instances, W1, b1, W2, b2):
    b2_value = float(np.asarray(b2).reshape(-1)[0])
    in_maps = make_in_maps(x, n_instances, W1, b1, W2, b2)
    res = run_spmd(in_maps, b2_value=b2_value)
    return np.concatenate([res.results[c]["out"] for c in range(N_CORES)], axis=0)



# revision 7
# speedup vs baseline: 2.3834x; 2.3834x over previous
"""Trainium2 Bass kernel for nn_BagModel_3d (segment_reduce).

Computation (per bag b):
  out[b] = (1/n_b) * sum_{i < n_b} relu(x[b, i, :] @ W1 + b1) @ W2 + b2

Strategy (8 cores, data-parallel over bags):
  * Host: sort bags by n_instances, snake-deal across cores (balanced work),
    concatenate ONLY the valid instances per core (exact compaction - the
    random n_b average ~256/512, so this halves DMA and matmul work), cast
    to bf16, zero-pad to G*128 columns.
  * Device, instance-major layout: for each 128-instance group, the x tile
    [d_in=128, inst=128] is the matmul STATIONARY operand and W1 [128, 256]
    the moving one, giving z^T [inst, dh] in PSUM.  The PSUM drain is then a
    bag-agnostic relu at FD=1024 (alternating ScalarE / VectorE) into a bf16
    h^T in SBUF - no per-bag accum pieces on the slow engines.
  * The ragged per-bag segment sum runs on TensorE: a {0,1} indicator matrix
    S[g] [128 inst, 32 bags] as stationary, h^T as moving, 4-way column
    tiling (tile_position), accumulating 4 PSUM band rows across all groups.
  * Final: one tensor_tensor_reduce contracts the pooled bands with W2 along
    the free axis, a [128,32] fold matmul adds the 4 bands per bag, then
    scale by 1/n_b and add b2 in a single tensor_scalar.
  * PE warmup: dummy matmuls during the initial DMA window so the HAM clock
    gate reaches 2.4 GHz before the real matmul stream starts.

b1 general-path note: padded columns are zeros and excluded by S, so no
relu(b1) correction is ever needed.  A nonzero b1 is folded in via an extra
K=1 matmul per group (ones-row x b1-row); the spec's b1 is all-zero so the
compiled program skips it.
"""
import os
import sys
import numpy as np

sys.path.insert(0, '/opt/trn_rl_repo')

# debug knobs (default = full-featured kernel)
DBG_NOWARM = os.environ.get('KDBG_NOWARM', '0') == '1'
DBG_NOTILEPOS = os.environ.get('KDBG_NOTILEPOS', '0') == '1'
DBG_NOPASS2 = os.environ.get('KDBG_NOPASS2', '0') == '1'
DBG_ALLSCALAR = os.environ.get('KDBG_ALLSCALAR', '0') == '1'
DBG_NOFINAL = os.environ.get('KDBG_NOFINAL', '0') == '1'
DBG_NODRAIN = os.environ.get('KDBG_NODRAIN', '0') == '1'

B, N_MAX, D_IN, D_H = 256, 512, 256, 256
N_CORES = 8
BAGS = B // N_CORES          # 32 bag slots per core
GPB = 4                      # instance groups per PSUM buffer (= col tiles)
N_WARM = 14                  # warmup matmuls (~3us cold PE time)

_PROGRAMS = {}


def _build_program(G, b1_nonzero):
    import concourse.bacc as bacc
    import concourse.tile as tile
    from concourse import mybir

    f32 = mybir.dt.float32
    bf16 = mybir.dt.bfloat16
    Alu = mybir.AluOpType
    Act = mybir.ActivationFunctionType

    W = G * 128
    NBLK = G // GPB

    nc = bacc.Bacc("TRN2", target_bir_lowering=False, debug=False)

    xt = nc.dram_tensor("xt", [D_IN, W], bf16, kind="ExternalInput").ap()
    w1 = nc.dram_tensor("w1", [D_IN, D_H], bf16, kind="ExternalInput").ap()
    s_t = nc.dram_tensor("s_t", [128, G * BAGS], bf16, kind="ExternalInput").ap()
    w2b = nc.dram_tensor("w2b", [128, D_H], f32, kind="ExternalInput").ap()
    fold = nc.dram_tensor("fold", [128, BAGS], f32, kind="ExternalInput").ap()
    inv = nc.dram_tensor("inv", [BAGS, 1], f32, kind="ExternalInput").ap()
    bias2 = nc.dram_tensor("bias2", [BAGS, 1], f32, kind="ExternalInput").ap()
    if b1_nonzero:
        b1r = nc.dram_tensor("b1r", [1, D_H], bf16, kind="ExternalInput").ap()
    out = nc.dram_tensor("out", [BAGS, 1], f32, kind="ExternalOutput").ap()

    with tile.TileContext(nc) as tc:
        with (
            tc.tile_pool(name="const", bufs=1) as cpool,
            tc.tile_pool(name="xsb", bufs=1) as xpool,
            tc.tile_pool(name="hsb", bufs=1) as hpool,
            tc.tile_pool(name="z", bufs=3, space="PSUM") as zpool,
            tc.tile_pool(name="sps", bufs=1, space="PSUM") as spool,
        ):
            # ---- PE warmup: zeros matmuls fill the HAM activity window ----
            warm = cpool.tile([128, 256], bf16, tag="warm")
            nc.vector.memset(warm[:], 0.0)
            small = spool.tile([128, 512], f32, tag="small")
            bands = small[:, 0:D_H]            # 4 x 32 bag-band rows
            pot = spool.tile([BAGS, 1], f32, tag="pot")
            po = pot[:]
            for _ in range(0 if DBG_NOWARM else N_WARM):
                nc.tensor.matmul(small[:, 256:512], warm[:, 0:128], warm[:],
                                 start=True, stop=True, skip_group_check=True)

            # ---- constants + x prefetch (HWDGE, bf16 host-cast) ----
            w1k0 = cpool.tile([128, D_H], bf16, tag="w1k0")
            w1k1 = cpool.tile([128, D_H], bf16, tag="w1k1")
            nc.sync.dma_start(w1k0[:], w1[0:128, :])
            nc.sync.dma_start(w1k1[:], w1[128:256, :])
            xk0 = xpool.tile([128, W], bf16, tag="xk0")
            xk1 = xpool.tile([128, W], bf16, tag="xk1")
            NCH = 8
            cw = W // NCH
            assert cw * NCH == W and cw % 128 == 0
            for ci in range(NCH):
                sl = slice(cw * ci, cw * (ci + 1))
                nc.sync.dma_start(xk0[:, sl], xt[0:128, sl])
                nc.sync.dma_start(xk1[:, sl], xt[128:256, sl])
            s_sb = cpool.tile([128, G * BAGS], bf16, tag="s_sb")
            nc.sync.dma_start(s_sb[:], s_t[:])
            w2sb = cpool.tile([128, D_H], f32, tag="w2sb")
            nc.sync.dma_start(w2sb[:], w2b[:])
            foldsb = cpool.tile([128, BAGS], f32, tag="foldsb")
            nc.sync.dma_start(foldsb[:], fold[:])
            invsb = cpool.tile([BAGS, 1], f32, tag="invsb")
            nc.sync.dma_start(invsb[:], inv[:])
            b2sb = cpool.tile([BAGS, 1], f32, tag="b2sb")
            nc.sync.dma_start(b2sb[:], bias2[:])
            if b1_nonzero:
                onesr = cpool.tile([1, 128], bf16, tag="onesr")
                nc.vector.memset(onesr[:], 1.0)
                b1sb = cpool.tile([1, D_H], bf16, tag="b1sb")
                nc.sync.dma_start(b1sb[:], b1r[:])

            hT = hpool.tile([128, G * D_H], bf16, tag="hT")

            # ---- main loop: GPB instance groups per PSUM buffer ----
            for blk in range(NBLK):
                zb = zpool.tile([128, GPB * D_H], f32, tag="zb")
                for j in range(GPB):
                    g = GPB * blk + j
                    zsl = zb[:, D_H * j:D_H * (j + 1)]
                    xsl = slice(128 * g, 128 * (g + 1))
                    nc.tensor.matmul(zsl, xk0[:, xsl], w1k0[:],
                                     start=True, stop=False)
                    nc.tensor.matmul(zsl, xk1[:, xsl], w1k1[:],
                                     start=False, stop=not b1_nonzero)
                    if b1_nonzero:
                        nc.tensor.matmul(zsl, onesr[:], b1sb[:],
                                         start=False, stop=True)
                # relu drain, whole buffer in one big-FD instruction
                hsl = hT[:, GPB * D_H * blk:GPB * D_H * (blk + 1)]
                if DBG_NODRAIN:
                    pass
                elif DBG_ALLSCALAR or blk % 2 == 0:
                    nc.scalar.activation(hsl, zb[:], Act.Relu,
                                         bias=0.0, scale=1.0)
                else:
                    nc.vector.tensor_scalar(hsl, zb[:], 0.0, None, op0=Alu.max)
                # segment-sum matmuls: indicator stationary, 4-way col tiling
                if not DBG_NOPASS2:
                    for j in range(GPB):
                        g = GPB * blk + j
                        kw = {}
                        if not DBG_NOTILEPOS:
                            kw['tile_position'] = (0, 32 * j)
                        nc.tensor.matmul(
                            bands[32 * j:32 * (j + 1), :],
                            s_sb[:, BAGS * g:BAGS * (g + 1)],
                            hT[:, D_H * g:D_H * (g + 1)],
                            start=(blk == 0), stop=(blk == NBLK - 1),
                            skip_group_check=True, **kw)

            # ---- W2 contraction + band fold + mean + bias ----
            scr = cpool.tile([128, D_H], f32, tag="scr")
            acc = cpool.tile([128, 1], f32, tag="acc")
            osb = cpool.tile([BAGS, 1], f32, tag="osb")
            if DBG_NOFINAL:
                nc.vector.memset(osb[:], 0.0)
            else:
                # acc[p] = sum_dh bands[p, dh] * W2[dh]  (proven stt+accum form)
                nc.vector.scalar_tensor_tensor(
                    scr[:], bands, 0.0, w2sb[:], op0=Alu.add, op1=Alu.mult,
                    accum_out=acc[:])
                nc.tensor.matmul(po, foldsb[:], acc[:], start=True, stop=True,
                                 skip_group_check=True)
                nc.vector.tensor_scalar(osb[:], po, invsb[:, 0:1], None,
                                        op0=Alu.mult)
                nc.vector.tensor_add(osb[:], osb[:], b2sb[:])
            nc.sync.dma_start(out[:], osb[:])

    nc.compile()
    return nc


def get_program(G, b1_nonzero):
    key = (int(G), bool(b1_nonzero))
    if key not in _PROGRAMS:
        _PROGRAMS[key] = _build_program(*key)
    return _PROGRAMS[key]


def _plan(n):
    """Snake-deal bags (sorted by size, desc) across cores; return
    assignment[core][slot] -> bag id and G (shared group count)."""
    order = np.argsort(-n, kind='stable')
    assignment = np.empty((N_CORES, BAGS), dtype=np.int64)
    for i, bag in enumerate(order):
        r, p = divmod(i, N_CORES)
        core = p if (r % 2 == 0) else (N_CORES - 1 - p)
        assignment[core, r] = bag
    v_max = max(int(n[assignment[c]].sum()) for c in range(N_CORES))
    G = -(-v_max // 128)
    G = -(-G // GPB) * GPB          # multiple of GPB (psum buffer / col tiles)
    return assignment, G


def make_in_maps(x, n_instances, W1, b1, W2, b2=None):
    import ml_dtypes
    bf16 = ml_dtypes.bfloat16

    x = np.asarray(x, dtype=np.float32)
    n = np.asarray(n_instances, dtype=np.int64)
    W1 = np.asarray(W1, dtype=np.float32)
    b1 = np.asarray(b1, dtype=np.float32).reshape(-1)
    W2 = np.asarray(W2, dtype=np.float32).reshape(-1)
    b2v = float(np.asarray(b2).reshape(-1)[0]) if b2 is not None else 0.0

    assignment, G = _plan(n)
    W = G * 128
    b1_nonzero = bool(np.any(b1 != 0.0))

    xflat = x.reshape(B * N_MAX, D_IN)
    w1_bf = np.ascontiguousarray(W1.astype(bf16))
    w2b = np.ascontiguousarray(
        np.broadcast_to(W2.reshape(1, D_H), (128, D_H)).astype(np.float32))
    foldm = np.zeros((128, BAGS), dtype=np.float32)
    foldm[np.arange(128), np.arange(128) % BAGS] = 1.0

    in_maps = []
    for c in range(N_CORES):
        bags = assignment[c]
        ns = n[bags]
        v = int(ns.sum())
        # gather valid instance rows: bag-major, instance-minor
        idx = np.concatenate(
            [bags[s] * N_MAX + np.arange(ns[s]) for s in range(BAGS)])
        xt = np.zeros((D_IN, W), dtype=bf16)
        xt[:, :v] = xflat[idx].T.astype(bf16)
        # indicator S: [W, 32] -> [128, G*32]
        starts = np.zeros(BAGS + 1, dtype=np.int64)
        np.cumsum(ns, out=starts[1:])
        s_full = np.zeros((W, BAGS), dtype=bf16)
        for s in range(BAGS):
            s_full[starts[s]:starts[s + 1], s] = bf16(1.0)
        s_t = np.ascontiguousarray(
            s_full.reshape(G, 128, BAGS).transpose(1, 0, 2).reshape(128, G * BAGS))
        im = {
            "xt": xt,
            "w1": w1_bf,
            "s_t": s_t,
            "w2b": w2b,
            "fold": foldm,
            "inv": (1.0 / ns.astype(np.float32)).reshape(BAGS, 1),
            "bias2": np.full((BAGS, 1), b2v, dtype=np.float32),
        }
        if b1_nonzero:
            im["b1r"] = np.ascontiguousarray(b1.reshape(1, D_H).astype(bf16))
        in_maps.append(im)
    return in_maps


def run_spmd(in_maps, b2_value=0.0, trace=False, **kwargs):
    from concourse import bass_utils
    if trace:
        # no S3 in this environment; keep trace artifacts local
        bass_utils.upload_artifacts = lambda tmpdir: tmpdir
    G = in_maps[0]["xt"].shape[1] // 128
    nc = get_program(G, "b1r" in in_maps[0])
    return bass_utils.run_bass_kernel_spmd(
        nc, in_maps, core_ids=list(range(N_CORES)), trace=trace, **kwargs)


def kernel(x, n_instances, W1, b1, W2, b2):
    n = np.asarray(n_instances, dtype=np.int64)
    assignment, _ = _plan(n)
    in_maps = make_in_maps(x, n_instances, W1, b1, W2, b2)
    res = run_spmd(in_maps)
    out = np.empty((B, 1), dtype=np.float32)
    for c in range(N_CORES):
        out[assignment[c]] = np.asarray(res.results[c]["out"],
                                        dtype=np.float32).reshape(BAGS, 1)
    return out
